# revision 1
# baseline (speedup 1.0000x reference)
"""Dynamic depthwise-conv branch (DynamicConvBranch) Trainium2 kernel.

Problem (hardcoded shapes):
  x  [16, 32, 384, 384] f32
  w1 [32, 128], b1 [128], w2 [128, 288], b2 [288]
  out[b,c] = conv2d_same3x3(x[b,c], k[b,c]) where
  k = reshape(relu(mean_hw(x) @ w1 + b1) @ w2 + b2, [B, 32, 3, 3])

Strategy: pure data parallel over batch (2 samples per core, 8 cores).
Per sample, x is held resident in SBUF as 4 row-strips x 32 channels of
[98, 386] tiles (1-row halos, zero-padded W edges).  Channel means are
computed with TensorE (ones-vector matmuls accumulated in PSUM), the
kernel-generator MLP runs as two small matmuls, and the depthwise 3x3
conv is computed as 3 PSUM-accumulated matmuls per (channel, strip)
using per-channel tridiagonal band matrices (built on VectorE from
host-baked 0/1 diagonal masks scaled by the generated kernel values).
"""

import numpy as np

B, C, H, W = 16, 32, 384, 384
NK = 32
HID = 128
KK = 3
N_CORES = 8
B_PER_CORE = B // N_CORES

GC = 8           # channels per DMA group
NG = C // GC     # 4 groups
SH = 96          # output rows per strip
NS = H // SH     # 4 strips
KP = SH + 2      # input rows per strip tile (with halo) = 98
WP = W + 2       # padded width: cols 0 and 385 are zero

_CACHE = {}


def _build_nc():
    from contextlib import ExitStack
    from concourse import bass, bacc, tile
    from concourse.bass import mybir

    f32 = mybir.dt.float32
    f32r = mybir.dt.float32r
    Alu = mybir.AluOpType
    Act = mybir.ActivationFunctionType

    nc = bacc.Bacc()

    x_d = nc.dram_tensor("x", [B_PER_CORE, C, H, W], f32r, kind="ExternalInput")
    w1_d = nc.dram_tensor("w1", [C, HID], f32, kind="ExternalInput")
    b1_d = nc.dram_tensor("b1", [HID], f32, kind="ExternalInput")
    w2_d = nc.dram_tensor("w2", [HID, NK * KK * KK], f32, kind="ExternalInput")
    b2_d = nc.dram_tensor("b2", [NK * KK * KK], f32, kind="ExternalInput")
    out_d = nc.dram_tensor("out", [B_PER_CORE, NK, H, W], f32, kind="ExternalOutput")

    # Host-baked diagonal masks: masks[dh][p, m] = 1 iff p == m + dh.
    # A band matrix A[p, m] = k[dh = p - m] is then
    #   A = k0*masks[0] + k1*masks[1] + k2*masks[2].
    import ml_dtypes
    masks_np = np.zeros((KP, KK, SH), dtype=np.float32)
    for dh in range(KK):
        for m in range(SH):
            masks_np[m + dh, dh, m] = 1.0
    masks_d = nc.inline_tensor(masks_np.astype(ml_dtypes.float8_e4m3), name="bandmasks")
    onesp_np = np.ones((KP, 1), dtype=np.float32)
    onesp_np[0, 0] = 0.0
    onesp_np[KP - 1, 0] = 0.0
    onesp_d = nc.inline_tensor(onesp_np, name="onesp")
    zrow_d = nc.inline_tensor(np.zeros((1, GC * WP), dtype=np.float32), name="zrow")
    zcol_d = nc.inline_tensor(np.zeros((KP, GC, 2), dtype=np.float32), name="zcol")

    with tile.TileContext(nc) as tc, ExitStack() as ctx:
        xpool_s0 = ctx.enter_context(tc.tile_pool(name="xs0", bufs=NG))
        xpool_mid = ctx.enter_context(tc.tile_pool(name="xmid", bufs=NG * 2))
        xpool_s3 = ctx.enter_context(tc.tile_pool(name="xs3", bufs=NG))
        cpool = ctx.enter_context(tc.tile_pool(name="const", bufs=1))
        mpool = ctx.enter_context(tc.tile_pool(name="mlp", bufs=1))
        apool = ctx.enter_context(tc.tile_pool(name="amat", bufs=5))
        cspool = ctx.enter_context(tc.tile_pool(name="csum", bufs=NS * NG + 2))
        opool = ctx.enter_context(tc.tile_pool(name="ostage", bufs=5))
        pp_ps = ctx.enter_context(
            tc.tile_pool(name="poolps", bufs=1, space=bass.MemorySpace.PSUM))
        kb_ps_pool = ctx.enter_context(
            tc.tile_pool(name="kbps", bufs=1, space=bass.MemorySpace.PSUM))
        cv_ps = ctx.enter_context(
            tc.tile_pool(name="convps", bufs=3, space=bass.MemorySpace.PSUM))

        # --- one-time constants ---
        cmrg = cpool.tile([KP, KK * SH + 4], mybir.dt.float8e4)
        masks = cmrg[:, 0:KK * SH].rearrange("p (k s) -> p k s", k=KK)
        nc.sync.dma_start(masks, masks_d[:])
        onesp = cmrg[:, KK * SH:KK * SH + 4].bitcast(f32r)
        nc.sync.dma_start(onesp, onesp_d[:].bitcast(f32r))

        ones1 = cpool.tile([1, HID], f32)         # for partition broadcast
        nc.vector.memset(ones1[:], 1.0)

        w1b = cpool.tile([C + 1, HID], f32)       # [w1; b1]
        nc.sync.dma_start(w1b[0:C, :], w1_d[:])
        nc.sync.dma_start(w1b[C:C + 1, :], b1_d[:].unsqueeze(0))

        w2s = cpool.tile([HID, NK * KK * KK], f32)
        nc.sync.dma_start(w2s[:], w2_d[:])
        b2s = cpool.tile([1, NK * KK * KK], f32)

        # tiny warm-up matmul: absorbs the PE preamble wait + const DMA lane
        # ticks so real matmuls carry few semaphore waits (ISA slot limit).
        warm_ps = pp_ps.tile([1, 1], f32, tag="pool")
        nc.tensor.matmul(warm_ps[:], onesp.bitcast(f32), onesp.bitcast(f32),
                         start=True, stop=True)

        for b in range(B_PER_CORE):
            # --- load x resident (8 channels per DMA) + pooled sums on PE ---
            nc.sync.dma_start(b2s[:], b2_d[:].unsqueeze(0))
            mlpx = mpool.tile([HID, 2 + C + 2], f32, tag="mlpx")
            h1s = mlpx[:, 0:1]
            pm = mlpx[0:C + 1, 1:2]
            pmrow = mlpx[0:1, 2:2 + C]
            prow_ps = pp_ps.tile([1, C], f32, tag="pool")
            xt = {}
            csum = {}
            for g in range(NG):
                c0 = g * GC
                for s in range(NS):
                    r0 = s * SH
                    xsrc = x_d[b, c0:c0 + GC]
                    if s == 0:
                        t = xpool_s0.tile([KP, GC, WP], f32r, tag="xs0")
                        if b == 0:
                            nc.sync.dma_start(
                                t[0:1, :, :], zrow_d[:].bitcast(f32r).rearrange(
                                    "p (c w) -> p c w", c=GC))  # row -1 = pad
                        nc.sync.dma_start(
                            t[1:KP, :, 1:W + 1],
                            xsrc[:, 0:KP - 1, :].rearrange("c r w -> r c w"))
                    elif s == NS - 1:
                        t = xpool_s3.tile([KP, GC, WP], f32r, tag="xs3")
                        if b == 0:
                            nc.sync.dma_start(
                                t[KP - 1:KP, :, :], zrow_d[:].bitcast(f32r).rearrange(
                                    "p (c w) -> p c w", c=GC))  # row H = pad
                        nc.sync.dma_start(
                            t[0:KP - 1, :, 1:W + 1],
                            xsrc[:, r0 - 1:H, :].rearrange("c r w -> r c w"))
                    else:
                        t = xpool_mid.tile([KP, GC, WP], f32r, tag="xmid")
                        nc.sync.dma_start(
                            t[:, :, 1:W + 1],
                            xsrc[:, r0 - 1:r0 + KP - 1, :].rearrange("c r w -> r c w"))
                    if b == 0:
                        # zero the W-pad columns (0 and 385); loads never
                        # touch them, so slot reuse keeps them zero
                        nc.sync.dma_start(t[:, :, 0:1],
                                          zcol_d[:, :, 0:1].bitcast(f32r))
                        nc.sync.dma_start(t[:, :, WP - 1:WP],
                                          zcol_d[:, :, 1:2].bitcast(f32r))
                    cs = cspool.tile([KP, GC], f32, tag="cs")
                    nc.vector.tensor_reduce(cs[:], t[:, :, :],
                                            mybir.AxisListType.X, Alu.add)
                    csum[(g, s)] = cs
                    xt[(g, s)] = t
                for s in range(NS):
                    nc.tensor.matmul(prow_ps[0:1, c0:c0 + GC], onesp.bitcast(f32),
                                     csum[(g, s)][:],
                                     start=(s == 0), stop=(s == NS - 1))

            # --- kernel-generator MLP ---
            nc.scalar.activation(pmrow, prow_ps[:], Act.Copy)
            pmt_ps = pp_ps.tile([C, 1], f32, tag="pool")
            nc.tensor.matmul(pmt_ps[:], pmrow, ones1[0:1, 0:1],
                             start=True, stop=True)
            nc.scalar.activation(pm[0:C, :], pmt_ps[:], Act.Copy,
                                 scale=1.0 / (H * W))
            nc.vector.memset(pm[C:C + 1, :], 1.0)

            h1_ps = pp_ps.tile([HID, 1], f32, tag="pool")
            nc.tensor.matmul(h1_ps[:], w1b[:], pm, start=True, stop=True)
            nc.scalar.activation(h1s, h1_ps[:], Act.Relu)

            k_ps = pp_ps.tile([1, NK * KK * KK], f32, tag="pool")
            nc.tensor.matmul(k_ps[:], h1s, w2s[:], start=True, stop=True)
            nc.vector.tensor_tensor(b2s[:], k_ps[:], b2s[:], Alu.add)

            kb = kb_ps_pool.tile([HID, NK * KK * KK], f32, tag="kbps")
            nc.tensor.matmul(kb[:], ones1[:], b2s[:], start=True, stop=True)

            # --- depthwise conv: band-matrix matmuls per (channel, strip) ---
            for c in range(C):
                g, cc = divmod(c, GC)
                amat = []
                for dw in range(KK):
                    a = apool.tile([KP, SH], f32r, tag="amat")
                    amat.append(a)
                    ks = lambda dh: kb[0:KP, c * 9 + dh * 3 + dw:c * 9 + dh * 3 + dw + 1]
                    nc.vector.tensor_scalar(a[:], masks[:, 0, :], ks(0), None,
                                            op0=Alu.mult)
                    nc.vector.scalar_tensor_tensor(a[:], masks[:, 1, :], ks(1),
                                                   a[:], op0=Alu.mult, op1=Alu.add)
                    nc.vector.scalar_tensor_tensor(a[:], masks[:, 2, :], ks(2),
                                                   a[:], op0=Alu.mult, op1=Alu.add)
                for j in range(NS // 2):
                    o_ps = cv_ps.tile([SH, 2, 512], f32, tag="cv")  # 2 banks
                    for half in range(2):
                        s = 2 * j + half
                        t = xt[(g, s)]
                        for dw in range(KK):
                            nc.tensor.matmul(o_ps[:, half, 0:W], amat[dw][:],
                                             t[:, cc, dw:dw + W],
                                             start=(dw == 0), stop=(dw == KK - 1))
                    for half in range(2):
                        s = 2 * j + half
                        ob = opool.tile([SH, W], f32, tag="ob")
                        nc.scalar.activation(ob[:], o_ps[:, half, 0:W], Act.Copy)
                        nc.sync.dma_start(out_d[b, c, s * SH:(s + 1) * SH, :],
                                          ob[:])

    nc.compile()
    return nc



def _make_exec():
    """Build + jit the SPMD executable once; returns a callable over numpy inputs."""
    import jax
    from jax.sharding import Mesh, PartitionSpec
    from jax.experimental.shard_map import shard_map
    from concourse import bass2jax
    import concourse.mybir as mybir

    nc = _build_nc()
    _CACHE["nc"] = nc
    bass2jax.install_neuronx_cc_hook()

    in_names, out_names, out_shapes, out_dtypes = [], [], [], []
    for alloc in nc.m.functions[0].allocations:
        if not isinstance(alloc, mybir.MemoryLocationSet):
            continue
        name = alloc.memorylocations[0].name
        if alloc.kind == "ExternalInput":
            in_names.append(name)
        elif alloc.kind == "ExternalOutput":
            out_names.append(name)
            out_shapes.append(tuple(alloc.tensor_shape))
            out_dtypes.append(mybir.dt.np(alloc.dtype))
    partition_name = nc.partition_id_tensor.name if nc.partition_id_tensor else None
    if partition_name in in_names:
        in_names.remove(partition_name)
    n_params = len(in_names)
    out_avals = [jax.core.ShapedArray(s, d) for s, d in zip(out_shapes, out_dtypes)]
    all_names = in_names + out_names
    if partition_name is not None:
        all_names = all_names + [partition_name]
    donate = tuple(range(n_params, n_params + len(out_names)))

    def _body(*args):
        operands = list(args)
        if partition_name is not None:
            operands.append(bass2jax.partition_id_tensor())
        outs = bass2jax._bass_exec_p.bind(
            *operands,
            out_avals=tuple(out_avals),
            in_names=tuple(all_names),
            out_names=tuple(out_names),
            lowering_input_output_aliases=(),
            sim_require_finite=True,
            sim_require_nnan=True,
            nc=nc,
        )
        return tuple(outs)

    devices = jax.devices()[:N_CORES]
    mesh = Mesh(np.asarray(devices), ("core",))
    in_specs = (PartitionSpec("core"),) * (n_params + len(out_names))
    out_specs = (PartitionSpec("core"),) * len(out_names)
    sharded = jax.jit(
        shard_map(_body, mesh=mesh, in_specs=in_specs, out_specs=out_specs,
                  check_rep=False),
        donate_argnums=donate, keep_unused=True)

    def run(in_maps):
        concat_in = [
            np.concatenate([np.asarray(in_maps[c][nm]) for c in range(N_CORES)], axis=0)
            for nm in in_names
        ]
        concat_zeros = [
            np.zeros((N_CORES * s[0], *s[1:]), d)
            for s, d in zip(out_shapes, out_dtypes)
        ]
        out_arrs = sharded(*concat_in, *concat_zeros)
        out_arrs = jax.block_until_ready(out_arrs)
        return {nm: np.asarray(out_arrs[i]) for i, nm in enumerate(out_names)}

    return run


def _run(inputs, trace=False):
    if "exec" not in _CACHE:
        _CACHE["exec"] = _make_exec()
    run = _CACHE["exec"]

    x = np.ascontiguousarray(inputs["x"], dtype=np.float32)
    in_maps = []
    for i in range(N_CORES):
        in_maps.append({
            "x": x[i * B_PER_CORE:(i + 1) * B_PER_CORE],
            "w1": inputs["w1"], "b1": inputs["b1"],
            "w2": inputs["w2"], "b2": inputs["b2"],
        })
    outs = run(in_maps)
    out = outs["out"].reshape(B, NK, H, W)
    return out, None


def kernel(**inputs):
    out, _ = _run(inputs, trace=False)
    return out



# revision 82
# speedup vs baseline: 1.4469x; 1.4469x over previous
"""Dynamic depthwise-conv branch (DynamicConvBranch) Trainium2 kernel.

Problem (hardcoded shapes):
  x  [16, 32, 384, 384] f32
  w1 [32, 128], b1 [128], w2 [128, 288], b2 [288]
  out[b,c] = conv2d_same3x3(x[b,c], k[b,c]) where
  k = reshape(relu(mean_hw(x) @ w1 + b1) @ w2 + b2, [B, 32, 3, 3])

Strategy: pure data parallel over batch (2 samples per core, 8 cores).
x is staged to bf16 on the host (halves load traffic; rel-err budget is
2e-2 and bf16 end-to-end lands ~5e-3).  Per sample, x is resident in
SBUF as 4 row-strips x 4 channel-groups of [98, 8, 386] bf16 tiles.
Channel means: DVE/GpSimd reduces per tile (rows 1..96 only, so halo /
uninitialized rows are never read) accumulated with ones-vector
matmuls in PSUM.  The 3x3 depthwise conv runs as 3 PSUM-accumulated
bf16 matmuls per (channel, strip) using per-channel tridiagonal band
matrices built on DVE (bf16 masks x kernel scalars, 4x DVE mode).
Edge strips drop the zero halo row by shortening the band matrix
(amat[1:], amat[:-1]) instead of loading zero rows.  PSUM is drained
by ACT into bf16 staging tiles; stores go out as one DMA per 2
channels, issued from ACT so they never block the SP load queue.
"""

import numpy as np

B, C, H, W = 16, 32, 384, 384
NK = 32
HID = 128
KK = 3
N_CORES = 8
B_PER_CORE = B // N_CORES

GC = 8           # channels per DMA group
NG = C // GC     # 4 groups
SH = 96          # output rows per strip
NS = H // SH     # 4 strips
KP = SH + 2      # tile partitions (1-row halo each side) = 98
WP = W + 2       # padded width: cols 0 and 385 are zero
XB_S0 = 6        # x tile slots, first-strip class (4 = one sample resident)
XB_MID = 12      # x tile slots, mid-strip class (8 = one sample resident)
XB_S3 = 6        # x tile slots, last-strip class

_CACHE = {}


def _build_nc():
    from contextlib import ExitStack
    from concourse import bass, bacc, tile
    from concourse.bass import mybir
    import ml_dtypes

    f32 = mybir.dt.float32
    bf16 = mybir.dt.bfloat16
    Alu = mybir.AluOpType
    Act = mybir.ActivationFunctionType

    nc = bacc.Bacc()

    x_d = nc.dram_tensor("x", [B_PER_CORE, C, H, W], bf16, kind="ExternalInput")
    w1_d = nc.dram_tensor("w1", [C, HID], f32, kind="ExternalInput")
    b1_d = nc.dram_tensor("b1", [HID], f32, kind="ExternalInput")
    w2_d = nc.dram_tensor("w2", [HID, NK * KK * KK], f32, kind="ExternalInput")
    b2_d = nc.dram_tensor("b2", [NK * KK * KK], f32, kind="ExternalInput")
    out_d = nc.dram_tensor("out", [B_PER_CORE, NK, H, W], bf16,
                           kind="ExternalOutput")

    # Host-baked diagonal masks replicated over dw: masks3[p, dh, m, dw] = 1
    # iff p == m + dh.  Band matrices for a whole channel-group build as
    # A3[p, c, m, dw] = sum_dh masks3[p, dh, m, dw] * k[c, dh, dw] with five
    # wide TensorTensor ops (the dw-last layout keeps every operand's last
    # AP dim packed, so DVE runs them in 2x mode).
    masks_np = np.zeros((KP, KK, SH, KK), dtype=np.float32)
    for dh in range(KK):
        for m in range(SH):
            masks_np[m + dh, dh, m, :] = 1.0
    masks_d = nc.inline_tensor(masks_np.astype(ml_dtypes.bfloat16),
                               name="bandmasks")
    zrow_d = nc.inline_tensor(np.zeros((1, GC, WP), dtype=ml_dtypes.bfloat16),
                              name="zrow")
    # strip-sum weights: drop the halo rows (compute ops must start at
    # partition 0, so pooling reads all 98 rows and the contraction masks)
    onesp_np = np.ones((KP, 1), dtype=np.float32)
    onesp_np[0, 0] = 0.0
    onesp_np[KP - 1, 0] = 0.0
    onesp_d = nc.inline_tensor(onesp_np, name="onesp")

    with tile.TileContext(nc) as tc, ExitStack() as ctx:
        xpool_s0 = ctx.enter_context(tc.tile_pool(name="xs0", bufs=XB_S0))
        xpool_mid = ctx.enter_context(tc.tile_pool(name="xmid", bufs=XB_MID))
        xpool_s3 = ctx.enter_context(tc.tile_pool(name="xs3", bufs=XB_S3))
        dpool = ctx.enter_context(tc.tile_pool(name="dump", bufs=2))
        cpool = ctx.enter_context(tc.tile_pool(name="const", bufs=1))
        mpool = ctx.enter_context(tc.tile_pool(name="mlp", bufs=1))
        apool = ctx.enter_context(tc.tile_pool(name="amat", bufs=4))
        tpool = ctx.enter_context(tc.tile_pool(name="atmp", bufs=2))
        cspool = ctx.enter_context(tc.tile_pool(name="csum", bufs=NS * NG + 2))
        opool = ctx.enter_context(tc.tile_pool(name="ostage", bufs=4))
        pp_ps = ctx.enter_context(
            tc.tile_pool(name="poolps", bufs=1, space=bass.MemorySpace.PSUM))
        kb_ps_pool = ctx.enter_context(
            tc.tile_pool(name="kbps", bufs=1, space=bass.MemorySpace.PSUM))
        cv_ps = ctx.enter_context(
            tc.tile_pool(name="convps", bufs=3, space=bass.MemorySpace.PSUM))

        # --- one-time constants ---
        masks = cpool.tile([KP, KK, SH, KK], bf16)
        nc.sync.dma_start(masks[:], masks_d[:])

        ones1 = cpool.tile([1, HID], f32)         # for partition broadcast
        nc.vector.memset(ones1[:], 1.0)
        onesp = cpool.tile([KP, 1], f32)          # strip-sum, halo rows zeroed
        nc.sync.dma_start(onesp[:], onesp_d[:])

        w1b = cpool.tile([C, HID], f32)
        nc.sync.dma_start(w1b[:], w1_d[:])
        b1row = cpool.tile([1, HID], f32)
        nc.sync.dma_start(b1row[:], b1_d[:].unsqueeze(0))

        w2s = cpool.tile([HID, NK * KK * KK], f32)
        nc.sync.dma_start(w2s[:], w2_d[:])
        b2row = cpool.tile([1, NK * KK * KK], f32)
        nc.sync.dma_start(b2row[:], b2_d[:].unsqueeze(0))

        # tiny warm-up matmul: absorbs the PE preamble wait + const DMA lane
        # ticks so real matmuls carry few semaphore waits (ISA slot limit).
        warm_ps = pp_ps.tile([1, 1], f32, tag="pool")
        nc.tensor.matmul(warm_ps[:], ones1[0:1, 0:1], ones1[0:1, 0:1],
                         start=True, stop=True)

        # x slots are zero-padded (cols 0/385, halo rows of edge strips) on
        # first use; loads never touch those regions, so the class-stable
        # slot rotation keeps them zero.
        x_acq = {"s0": 0, "mid": 0, "s3": 0}
        x_pools = {"s0": (xpool_s0, XB_S0), "mid": (xpool_mid, XB_MID),
                   "s3": (xpool_s3, XB_S3)}

        def x_tile(klass):
            pool, bufs = x_pools[klass]
            t = pool.tile([KP, GC, WP], bf16, tag=klass)
            if x_acq[klass] < bufs:
                nc.gpsimd.memset(t[:, :, 0:1], 0.0)
                nc.gpsimd.memset(t[:, :, WP - 1:WP], 0.0)
                if klass == "s0":
                    nc.sync.dma_start(t[0:1, :, :], zrow_d[:])
                elif klass == "s3":
                    nc.sync.dma_start(t[KP - 1:KP, :, :], zrow_d[:])
            x_acq[klass] += 1
            return t

        xt = {}      # (b, g, s) -> x tile
        csum = {}    # (b, g, s) -> [96, GC] f32 row sums

        def emit_load(b, g, eng=None):
            """Load DMAs for one channel-group of sample b."""
            eng = eng or nc.sync
            c0 = g * GC
            for s in range(NS):
                r0 = s * SH
                xsrc = x_d[b, c0:c0 + GC]
                if s == 0:
                    t = x_tile("s0")
                    eng.dma_start(
                        t[1:KP, :, 1:W + 1],
                        xsrc[:, 0:KP - 1, :].rearrange("c r w -> r c w"))
                elif s == NS - 1:
                    t = x_tile("s3")
                    eng.dma_start(
                        t[0:KP - 1, :, 1:W + 1],
                        xsrc[:, r0 - 1:H, :].rearrange("c r w -> r c w"))
                else:
                    t = x_tile("mid")
                    eng.dma_start(
                        t[:, :, 1:W + 1],
                        xsrc[:, r0 - 1:r0 + KP - 1, :].rearrange(
                            "c r w -> r c w"))
                xt[(b, g, s)] = t

        def pool_tile(b, g, s):
            """W-sums of one tile on DVE: tensor_scalar runs in 4x mode even
            with the per-partition accumulator attached (160ns/channel).
            Reads all 98 rows (partition offset must be 0); halo rows are
            dropped later by the onesp strip-sum contraction."""
            t = xt[(b, g, s)]
            cs = cspool.tile([KP, GC], f32, tag="cs")
            dump = dpool.tile([KP, W], bf16, tag="dump")
            for cc in range(GC):
                nc.vector.tensor_scalar(dump[:], t[:, cc, 1:W + 1],
                                        1.0, 0.0, op0=Alu.mult, op1=Alu.add,
                                        accum_out=cs[:, cc:cc + 1])
            csum[(b, g, s)] = cs

        def emit_mlp(b):
            """Strip-sum matmuls + kernel-generator MLP (f32); returns kb.
            Chain only touches PE and ACT (DVE may be backlogged)."""
            mlpx = mpool.tile([HID, 2 + C + 2], f32, tag="mlpx")
            h1s = mlpx[:, 0:1]
            pm = mlpx[0:C, 1:2]
            pmrow = mlpx[0:1, 2:2 + C]
            prow_ps = pp_ps.tile([1, C], f32, tag="pool")
            for g in range(NG):
                c0 = g * GC
                for s in range(NS):
                    nc.tensor.matmul(prow_ps[0:1, c0:c0 + GC], onesp[:],
                                     csum[(b, g, s)][:],
                                     start=(s == 0), stop=(s == NS - 1))
            nc.scalar.activation(pmrow, prow_ps[:], Act.Copy)
            pmt_ps = pp_ps.tile([C, 1], f32, tag="pool")
            nc.tensor.matmul(pmt_ps[:], pmrow, ones1[0:1, 0:1],
                             start=True, stop=True)
            nc.scalar.activation(pm, pmt_ps[:], Act.Copy,
                                 scale=1.0 / (H * W))

            h1_ps = pp_ps.tile([HID, 1], f32, tag="pool")
            nc.tensor.matmul(h1_ps[:], w1b[:], pm, start=True, stop=False)
            nc.tensor.matmul(h1_ps[:], b1row[:], ones1[0:1, 0:1],
                             start=False, stop=True)
            nc.scalar.activation(h1s, h1_ps[:], Act.Relu)

            k_ps = pp_ps.tile([1, NK * KK * KK], f32, tag="pool")
            nc.tensor.matmul(k_ps[:], h1s, w2s[:], start=True, stop=True)
            krow = mpool.tile([1, NK * KK * KK], f32, tag="krow")
            nc.scalar.activation(krow[:], k_ps[:], Act.Copy)

            kb = kb_ps_pool.tile([HID, NK * KK * KK], f32, tag="kbps")
            nc.tensor.matmul(kb[:], ones1[:], krow[:], start=True, stop=False)
            nc.tensor.matmul(kb[:], ones1[:], b2row[:], start=False, stop=True)
            kbs = mpool.tile([HID, NK * KK * KK], bf16, tag="kbs")
            nc.scalar.activation(kbs[:], kb[:], Act.Copy)
            # gated warm-up: PE idled through pooling, so its p-state clock
            # dropped; 12 back-to-back dummy matmuls (first ready only once
            # kbs exists) ramp it back to full speed exactly while DVE builds
            # the first band matrices, so conv starts at 2.4 GHz
            for _ in range(12):
                w_ps = pp_ps.tile([1, NK * KK * KK], f32, tag="pool",
                                  name="w_ps")
                nc.tensor.matmul(w_ps[:], kbs[0:1, 0:1], kbs[0:1, :],
                                 start=True, stop=True)
            return kbs

        def emit_amat_group(kbs, g, halves=1):
            """Band matrices for a whole 8-channel group in 5 wide DVE
            TensorTensor ops (2x mode: every operand's last AP dim is the
            packed dw axis): A3[p, c, m, dw] = sum_dh masks3 * k[c, dh, dw].
            halves=2 splits the build in two for lower first-channel latency.
            """
            a3 = apool.tile([KP, GC, SH, KK], bf16, tag="a3", name="a3")
            # [98, 8ch, 9] view of this group's kernel block
            gb = kbs[0:KP, g * GC * 9:(g + 1) * GC * 9].rearrange(
                "p (c z) -> p c z", c=GC)
            hc = GC // halves
            for h in range(halves):
                c0, c1 = h * hc, (h + 1) * hc
                for dh in range(KK):
                    m_b = masks[:, dh].unsqueeze(1).broadcast_to(
                        [KP, hc, SH, KK])
                    k_b = gb[:, c0:c1, dh * KK:(dh + 1) * KK].unsqueeze(2) \
                        .broadcast_to([KP, hc, SH, KK])
                    if dh == 0:
                        nc.vector.tensor_tensor(a3[:, c0:c1], m_b, k_b,
                                                Alu.mult)
                    else:
                        t = tpool.tile([KP, hc, SH, KK], bf16, tag="at",
                                       name="at")
                        nc.vector.tensor_tensor(t[:], m_b, k_b, Alu.mult)
                        nc.vector.tensor_tensor(a3[:, c0:c1], a3[:, c0:c1],
                                                t[:], Alu.add)
            return a3

        def emit_conv_channel(b, c, a3, ob):
            """12 PSUM-accumulated matmuls + drains for channel c."""
            g, cc = divmod(c, GC)
            for j in range(NS // 2):
                o_ps = cv_ps.tile([SH, 2, 512], f32, tag="cv")  # 2 banks
                for half in range(2):
                    s = 2 * j + half
                    t = xt[(b, g, s)]
                    for dw in range(KK):
                        nc.tensor.matmul(o_ps[:, half, 0:W],
                                         a3[:, cc, :, dw],
                                         t[:, cc, dw:dw + W],
                                         start=(dw == 0), stop=(dw == KK - 1))
                # PSUM drains: GpSimd cannot access PSUM on HW, so ACT
                # carries 7/8 and DVE (busy with amats+pooling) takes 1/8
                if (2 * c + j) % 8 == 1:
                    nc.vector.tensor_copy(ob[:, c % 2, 2 * j:2 * j + 2, :],
                                          o_ps[:, :, 0:W])
                else:
                    nc.scalar.activation(ob[:, c % 2, 2 * j:2 * j + 2, :],
                                         o_ps[:, :, 0:W], Act.Copy)
            if c % 2 == 1:
                # one store per channel pair, issued from ACT so the SP load
                # queue never waits behind store dependencies
                nc.scalar.dma_start(
                    out_d[b, c - 1:c + 1, :, :].rearrange(
                        "c (s p) w -> p c s w", s=NS),
                    ob[:])

        # ---------------- schedule ----------------
        # head: load + pool sample 0: DVE tensor-scalar accumulators for
        # most tiles, ACT's accumulator for one tile per group plus g3-s1
        # (ACT is otherwise idle in the head), all chasing the SP loads
        def act_pool_tile(b, g, s):
            t = xt[(b, g, s)]
            cs = cspool.tile([KP, GC], f32, tag="cs", name="cs")
            dmp = dpool.tile([KP, W], bf16, tag="dump", name="dump")
            for cc in range(GC):
                nc.scalar.activation(dmp[:], t[:, cc, 1:W + 1],
                                     Act.Copy, accum_out=cs[:, cc:cc + 1])
            csum[(b, g, s)] = cs

        # ACT only pools early-loaded tiles: its in-order queue must be free
        # for the MLP chain the moment the last (DVE-pooled) tile lands
        head_act = {(0, 2), (1, 1), (1, 3), (2, 2)}
        for g in range(NG):
            # alternate HWDGE (SP) and SWDGE (GpSimd) issue: two issue paths
            # keep the DMA pipe at its transfer-limited rate in the head
            emit_load(0, g)
            for s in range(NS):
                if (g, s) in head_act:
                    act_pool_tile(0, g, s)
                else:
                    pool_tile(0, g, s)
        kb0 = emit_mlp(0)

        # conv sample 0; sample 1's loads go out immediately in order
        # g2,g3,g0,g1 (g2/g3 land on fresh slots, g0/g1 self-gate on slot
        # reuse after conv0 drains channels 7/15).  Its pooling interleaves
        # into the DVE stream keyed to conv progress so the in-order stream
        # never stalls on an unloaded tile.  GpSimd builds every 4th
        # channel's band matrices to keep DVE under the PE-window budget.
        for g in (2, 3, 0, 1):
            emit_load(1, g)

        def conv_sample(b, kbs, interleave):
            ob = None
            last_a3 = None
            for g in range(NG):
                a3 = emit_amat_group(kbs, g, halves=2 if g == 0 else 1)
                last_a3 = a3
                for cc in range(GC):
                    c = g * GC + cc
                    if c % 2 == 0:
                        ob = opool.tile([SH, 2, NS, W], bf16, tag="ob",
                                        name="ob")
                    emit_conv_channel(b, c, a3, ob)
            if interleave:
                for g in (2, 3, 0, 1):
                    for s in range(NS):
                        if (g == 1 and s >= 2) or (g == 0 and s >= 2):
                            # late-loaded groups: ACT's accumulator reduce
                            act_pool_tile(1, g, s)
                        else:
                            pool_tile(1, g, s)

        conv_sample(0, kb0, True)
        kb1 = emit_mlp(1)
        conv_sample(1, kb1, False)

    nc.compile()
    return nc


def _make_exec():
    """Build + jit the SPMD executable once; returns a callable over numpy inputs."""
    import jax
    from jax.sharding import Mesh, PartitionSpec
    from jax.experimental.shard_map import shard_map
    from concourse import bass2jax
    import concourse.mybir as mybir

    nc = _build_nc()
    _CACHE["nc"] = nc
    bass2jax.install_neuronx_cc_hook()

    in_names, out_names, out_shapes, out_dtypes = [], [], [], []
    for alloc in nc.m.functions[0].allocations:
        if not isinstance(alloc, mybir.MemoryLocationSet):
            continue
        name = alloc.memorylocations[0].name
        if alloc.kind == "ExternalInput":
            in_names.append(name)
        elif alloc.kind == "ExternalOutput":
            out_names.append(name)
            out_shapes.append(tuple(alloc.tensor_shape))
            out_dtypes.append(mybir.dt.np(alloc.dtype))
    partition_name = nc.partition_id_tensor.name if nc.partition_id_tensor else None
    if partition_name in in_names:
        in_names.remove(partition_name)
    n_params = len(in_names)
    out_avals = [jax.core.ShapedArray(s, d) for s, d in zip(out_shapes, out_dtypes)]
    all_names = in_names + out_names
    if partition_name is not None:
        all_names = all_names + [partition_name]
    donate = tuple(range(n_params, n_params + len(out_names)))

    def _body(*args):
        operands = list(args)
        if partition_name is not None:
            operands.append(bass2jax.partition_id_tensor())
        outs = bass2jax._bass_exec_p.bind(
            *operands,
            out_avals=tuple(out_avals),
            in_names=tuple(all_names),
            out_names=tuple(out_names),
            lowering_input_output_aliases=(),
            sim_require_finite=True,
            sim_require_nnan=True,
            nc=nc,
        )
        return tuple(outs)

    devices = jax.devices()[:N_CORES]
    mesh = Mesh(np.asarray(devices), ("core",))
    in_specs = (PartitionSpec("core"),) * (n_params + len(out_names))
    out_specs = (PartitionSpec("core"),) * len(out_names)
    sharded = jax.jit(
        shard_map(_body, mesh=mesh, in_specs=in_specs, out_specs=out_specs,
                  check_rep=False),
        donate_argnums=donate, keep_unused=True)

    def run(in_maps):
        concat_in = [
            np.concatenate([np.asarray(in_maps[c][nm]) for c in range(N_CORES)], axis=0)
            for nm in in_names
        ]
        concat_zeros = [
            np.zeros((N_CORES * s[0], *s[1:]), d)
            for s, d in zip(out_shapes, out_dtypes)
        ]
        out_arrs = sharded(*concat_in, *concat_zeros)
        out_arrs = jax.block_until_ready(out_arrs)
        return {nm: np.asarray(out_arrs[i]) for i, nm in enumerate(out_names)}

    return run


def _run(inputs, trace=False):
    import ml_dtypes
    if "exec" not in _CACHE:
        _CACHE["exec"] = _make_exec()
    run = _CACHE["exec"]

    x16 = np.ascontiguousarray(inputs["x"]).astype(ml_dtypes.bfloat16)
    in_maps = []
    for i in range(N_CORES):
        in_maps.append({
            "x": x16[i * B_PER_CORE:(i + 1) * B_PER_CORE],
            "w1": inputs["w1"], "b1": inputs["b1"],
            "w2": inputs["w2"], "b2": inputs["b2"],
        })
    outs = run(in_maps)
    out = outs["out"].reshape(B, NK, H, W).astype(np.float32)
    return out, None


def kernel(**inputs):
    out, _ = _run(inputs, trace=False)
    return out


# revision 90
# speedup vs baseline: 1.6144x; 1.1157x over previous
"""Dynamic depthwise-conv branch (DynamicConvBranch) Trainium2 kernel.

Problem (hardcoded shapes):
  x  [16, 32, 384, 384] f32
  w1 [32, 128], b1 [128], w2 [128, 288], b2 [288]
  out[b,c] = conv2d_same3x3(x[b,c], k[b,c]) where
  k = reshape(relu(mean_hw(x) @ w1 + b1) @ w2 + b2, [B, 32, 3, 3])

Strategy: pure data parallel over batch (2 samples per core, 8 cores).
x and the output are staged to bf16 on the host (halves DMA traffic;
rel-err budget is 2e-2, measured end-to-end error ~5.7e-3).  Per
sample, x is resident in SBUF as 4 row-strips x 4 channel-groups of
[98, 8, 386] bf16 tiles (zero pad columns + halo rows kept clean by
class-stable slot rotation).  Channel means: one tensor_scalar with a
per-partition accumulator per (tile, channel) -- it runs in the DVE 4x
perf mode even with accum attached -- with a few tiles on ACT's
activation accumulator; halo rows are dropped by a zeroed ones-vector
in the PSUM strip-sum matmuls.  The kernel-generator MLP runs on
PE+ACT only, followed by kbs-gated dummy matmuls that re-ramp the PE
p-state while DVE builds the first band matrices.  Band matrices for
a whole 8-channel group build in 5 wide broadcast TensorTensor ops
(2x mode; dw-last layout).  The 3x3 depthwise conv is 3 PSUM-
accumulated bf16 matmuls per (channel, strip).  PSUM is drained 7/8
by ACT, 1/8 by DVE (GpSimd cannot access PSUM on hardware) into bf16
staging; stores go out as one DMA per 2 channels, issued from ACT so
they never block the SP load queue.  Sample 1's loads start during
sample 0's conv (fresh slots first: g2,g3), its pooling overlaps the
conv window, so the second conv window starts with only a short MLP
gap.
"""

import numpy as np

B, C, H, W = 16, 32, 384, 384
NK = 32
HID = 128
KK = 3
N_CORES = 8
B_PER_CORE = B // N_CORES

GC = 8           # channels per DMA group
NG = C // GC     # 4 groups
SH = 96          # output rows per strip
NS = H // SH     # 4 strips
KP = SH + 2      # tile partitions (1-row halo each side) = 98
WP = W + 2       # padded width: cols 0 and 385 are zero
XB_S0 = 6        # x tile slots, first-strip class (4 = one sample resident)
XB_MID = 12      # x tile slots, mid-strip class (8 = one sample resident)
XB_S3 = 6        # x tile slots, last-strip class

_CACHE = {}


def _build_nc():
    from contextlib import ExitStack
    from concourse import bass, bacc, tile
    from concourse.bass import mybir
    import ml_dtypes

    f32 = mybir.dt.float32
    bf16 = mybir.dt.bfloat16
    Alu = mybir.AluOpType
    Act = mybir.ActivationFunctionType

    nc = bacc.Bacc()

    x_d = nc.dram_tensor("x", [B_PER_CORE, C, H, W], bf16, kind="ExternalInput")
    w1_d = nc.dram_tensor("w1", [C, HID], f32, kind="ExternalInput")
    b1_d = nc.dram_tensor("b1", [HID], f32, kind="ExternalInput")
    w2_d = nc.dram_tensor("w2", [HID, NK * KK * KK], f32, kind="ExternalInput")
    b2_d = nc.dram_tensor("b2", [NK * KK * KK], f32, kind="ExternalInput")
    out_d = nc.dram_tensor("out", [B_PER_CORE, NK, H, W], bf16,
                           kind="ExternalOutput")

    # Host-baked diagonal masks replicated over dw: masks3[p, dh, m, dw] = 1
    # iff p == m + dh.  Band matrices for a whole channel-group build as
    # A3[p, c, m, dw] = sum_dh masks3[p, dh, m, dw] * k[c, dh, dw] with five
    # wide TensorTensor ops (the dw-last layout keeps every operand's last
    # AP dim packed, so DVE runs them in 2x mode).
    masks_np = np.zeros((KP, KK, SH, KK), dtype=np.float32)
    for dh in range(KK):
        for m in range(SH):
            masks_np[m + dh, dh, m, :] = 1.0
    masks_d = nc.inline_tensor(masks_np.astype(ml_dtypes.bfloat16),
                               name="bandmasks")
    zrow_d = nc.inline_tensor(np.zeros((1, GC, WP), dtype=ml_dtypes.bfloat16),
                              name="zrow")
    # strip-sum weights: drop the halo rows (compute ops must start at
    # partition 0, so pooling reads all 98 rows and the contraction masks)
    onesp_np = np.ones((KP, 1), dtype=np.float32)
    onesp_np[0, 0] = 0.0
    onesp_np[KP - 1, 0] = 0.0
    onesp_d = nc.inline_tensor(onesp_np, name="onesp")

    with tile.TileContext(nc) as tc, ExitStack() as ctx:
        xpool_s0 = ctx.enter_context(tc.tile_pool(name="xs0", bufs=XB_S0))
        xpool_mid = ctx.enter_context(tc.tile_pool(name="xmid", bufs=XB_MID))
        xpool_s3 = ctx.enter_context(tc.tile_pool(name="xs3", bufs=XB_S3))
        dpool = ctx.enter_context(tc.tile_pool(name="dump", bufs=2))
        cpool = ctx.enter_context(tc.tile_pool(name="const", bufs=1))
        mpool = ctx.enter_context(tc.tile_pool(name="mlp", bufs=1))
        apool = ctx.enter_context(tc.tile_pool(name="amat", bufs=4))
        tpool = ctx.enter_context(tc.tile_pool(name="atmp", bufs=2))
        cspool = ctx.enter_context(tc.tile_pool(name="csum", bufs=NS * NG + 2))
        opool = ctx.enter_context(tc.tile_pool(name="ostage", bufs=4))
        pp_ps = ctx.enter_context(
            tc.tile_pool(name="poolps", bufs=1, space=bass.MemorySpace.PSUM))
        kb_ps_pool = ctx.enter_context(
            tc.tile_pool(name="kbps", bufs=1, space=bass.MemorySpace.PSUM))
        cv_ps = ctx.enter_context(
            tc.tile_pool(name="convps", bufs=3, space=bass.MemorySpace.PSUM))

        # --- one-time constants ---
        masks = cpool.tile([KP, KK, SH, KK], bf16)
        nc.sync.dma_start(masks[:], masks_d[:])

        ones1 = cpool.tile([1, HID], f32)         # for partition broadcast
        nc.vector.memset(ones1[:], 1.0)
        onesp = cpool.tile([KP, 1], f32)          # strip-sum, halo rows zeroed
        nc.sync.dma_start(onesp[:], onesp_d[:])

        w1b = cpool.tile([C, HID], f32)
        nc.sync.dma_start(w1b[:], w1_d[:])
        b1row = cpool.tile([1, HID], f32)
        nc.sync.dma_start(b1row[:], b1_d[:].unsqueeze(0))

        w2s = cpool.tile([HID, NK * KK * KK], f32)
        nc.sync.dma_start(w2s[:], w2_d[:])
        b2row = cpool.tile([1, NK * KK * KK], f32)
        nc.sync.dma_start(b2row[:], b2_d[:].unsqueeze(0))

        # tiny warm-up matmul: absorbs the PE preamble wait + const DMA lane
        # ticks so real matmuls carry few semaphore waits (ISA slot limit).
        warm_ps = pp_ps.tile([1, 1], f32, tag="pool")
        nc.tensor.matmul(warm_ps[:], ones1[0:1, 0:1], ones1[0:1, 0:1],
                         start=True, stop=True)

        # x slots are zero-padded (cols 0/385, halo rows of edge strips) on
        # first use; loads never touch those regions, so the class-stable
        # slot rotation keeps them zero.
        x_acq = {"s0": 0, "mid": 0, "s3": 0}
        x_pools = {"s0": (xpool_s0, XB_S0), "mid": (xpool_mid, XB_MID),
                   "s3": (xpool_s3, XB_S3)}

        def x_tile(klass):
            pool, bufs = x_pools[klass]
            t = pool.tile([KP, GC, WP], bf16, tag=klass)
            if x_acq[klass] < bufs:
                nc.gpsimd.memset(t[:, :, 0:1], 0.0)
                nc.gpsimd.memset(t[:, :, WP - 1:WP], 0.0)
                if klass == "s0":
                    nc.sync.dma_start(t[0:1, :, :], zrow_d[:])
                elif klass == "s3":
                    nc.sync.dma_start(t[KP - 1:KP, :, :], zrow_d[:])
            x_acq[klass] += 1
            return t

        xt = {}      # (b, g, s) -> x tile
        csum = {}    # (b, g, s) -> [96, GC] f32 row sums

        def emit_load(b, g, eng=None):
            """Load DMAs for one channel-group of sample b."""
            eng = eng or nc.sync
            c0 = g * GC
            for s in range(NS):
                r0 = s * SH
                xsrc = x_d[b, c0:c0 + GC]
                if s == 0:
                    t = x_tile("s0")
                    eng.dma_start(
                        t[1:KP, :, 1:W + 1],
                        xsrc[:, 0:KP - 1, :].rearrange("c r w -> r c w"))
                elif s == NS - 1:
                    t = x_tile("s3")
                    eng.dma_start(
                        t[0:KP - 1, :, 1:W + 1],
                        xsrc[:, r0 - 1:H, :].rearrange("c r w -> r c w"))
                else:
                    t = x_tile("mid")
                    eng.dma_start(
                        t[:, :, 1:W + 1],
                        xsrc[:, r0 - 1:r0 + KP - 1, :].rearrange(
                            "c r w -> r c w"))
                xt[(b, g, s)] = t

        def pool_tile(b, g, s):
            """W-sums of one tile on DVE: tensor_scalar runs in 4x mode even
            with the per-partition accumulator attached (160ns/channel).
            Reads all 98 rows (partition offset must be 0); halo rows are
            dropped later by the onesp strip-sum contraction."""
            t = xt[(b, g, s)]
            cs = cspool.tile([KP, GC], f32, tag="cs")
            dump = dpool.tile([KP, W], bf16, tag="dump")
            for cc in range(GC):
                nc.vector.tensor_scalar(dump[:], t[:, cc, 1:W + 1],
                                        1.0, 0.0, op0=Alu.mult, op1=Alu.add,
                                        accum_out=cs[:, cc:cc + 1])
            csum[(b, g, s)] = cs

        def emit_mlp(b):
            """Strip-sum matmuls + kernel-generator MLP (f32); returns kb.
            Chain only touches PE and ACT (DVE may be backlogged)."""
            mlpx = mpool.tile([HID, 2 + C + 2], f32, tag="mlpx")
            h1s = mlpx[:, 0:1]
            pm = mlpx[0:C, 1:2]
            pmrow = mlpx[0:1, 2:2 + C]
            prow_ps = pp_ps.tile([1, C], f32, tag="pool")
            for g in range(NG):
                c0 = g * GC
                for s in range(NS):
                    nc.tensor.matmul(prow_ps[0:1, c0:c0 + GC], onesp[:],
                                     csum[(b, g, s)][:],
                                     start=(s == 0), stop=(s == NS - 1))
            nc.scalar.activation(pmrow, prow_ps[:], Act.Copy)
            pmt_ps = pp_ps.tile([C, 1], f32, tag="pool")
            nc.tensor.matmul(pmt_ps[:], pmrow, ones1[0:1, 0:1],
                             start=True, stop=True)
            nc.scalar.activation(pm, pmt_ps[:], Act.Copy,
                                 scale=1.0 / (H * W))

            h1_ps = pp_ps.tile([HID, 1], f32, tag="pool")
            nc.tensor.matmul(h1_ps[:], w1b[:], pm, start=True, stop=False)
            nc.tensor.matmul(h1_ps[:], b1row[:], ones1[0:1, 0:1],
                             start=False, stop=True)
            nc.scalar.activation(h1s, h1_ps[:], Act.Relu)

            k_ps = pp_ps.tile([1, NK * KK * KK], f32, tag="pool")
            nc.tensor.matmul(k_ps[:], h1s, w2s[:], start=True, stop=True)
            krow = mpool.tile([1, NK * KK * KK], f32, tag="krow")
            nc.scalar.activation(krow[:], k_ps[:], Act.Copy)

            kb = kb_ps_pool.tile([HID, NK * KK * KK], f32, tag="kbps")
            nc.tensor.matmul(kb[:], ones1[:], krow[:], start=True, stop=False)
            nc.tensor.matmul(kb[:], ones1[:], b2row[:], start=False, stop=True)
            kbs = mpool.tile([HID, NK * KK * KK], bf16, tag="kbs")
            nc.scalar.activation(kbs[:], kb[:], Act.Copy)
            # gated warm-up: PE idled through pooling, so its p-state clock
            # dropped; 12 back-to-back dummy matmuls (first ready only once
            # kbs exists) ramp it back to full speed exactly while DVE builds
            # the first band matrices, so conv starts at 2.4 GHz
            for _ in range(12):
                w_ps = pp_ps.tile([1, NK * KK * KK], f32, tag="pool",
                                  name="w_ps")
                nc.tensor.matmul(w_ps[:], kbs[0:1, 0:1], kbs[0:1, :],
                                 start=True, stop=True)
            return kbs

        def emit_amat_group(kbs, g, halves=1):
            """Band matrices for a whole 8-channel group in 5 wide DVE
            TensorTensor ops (2x mode: every operand's last AP dim is the
            packed dw axis): A3[p, c, m, dw] = sum_dh masks3 * k[c, dh, dw].
            halves=2 splits the build in two for lower first-channel latency.
            """
            a3 = apool.tile([KP, GC, SH, KK], bf16, tag="a3", name="a3")
            # [98, 8ch, 9] view of this group's kernel block
            gb = kbs[0:KP, g * GC * 9:(g + 1) * GC * 9].rearrange(
                "p (c z) -> p c z", c=GC)
            hc = GC // halves
            for h in range(halves):
                c0, c1 = h * hc, (h + 1) * hc
                for dh in range(KK):
                    m_b = masks[:, dh].unsqueeze(1).broadcast_to(
                        [KP, hc, SH, KK])
                    k_b = gb[:, c0:c1, dh * KK:(dh + 1) * KK].unsqueeze(2) \
                        .broadcast_to([KP, hc, SH, KK])
                    if dh == 0:
                        nc.vector.tensor_tensor(a3[:, c0:c1], m_b, k_b,
                                                Alu.mult)
                    else:
                        t = tpool.tile([KP, hc, SH, KK], bf16, tag="at",
                                       name="at")
                        nc.vector.tensor_tensor(t[:], m_b, k_b, Alu.mult)
                        nc.vector.tensor_tensor(a3[:, c0:c1], a3[:, c0:c1],
                                                t[:], Alu.add)
            return a3

        def emit_conv_channel(b, c, a3, ob):
            """12 PSUM-accumulated matmuls + drains for channel c."""
            g, cc = divmod(c, GC)
            for j in range(NS // 2):
                o_ps = cv_ps.tile([SH, 2, 512], f32, tag="cv")  # 2 banks
                for half in range(2):
                    s = 2 * j + half
                    t = xt[(b, g, s)]
                    for dw in range(KK):
                        nc.tensor.matmul(o_ps[:, half, 0:W],
                                         a3[:, cc, :, dw],
                                         t[:, cc, dw:dw + W],
                                         start=(dw == 0), stop=(dw == KK - 1))
                # PSUM drains all on ACT (GpSimd cannot access PSUM on
                # HW; DVE carries every band-matrix build and all pooling)
                nc.scalar.activation(ob[:, c % 2, 2 * j:2 * j + 2, :],
                                     o_ps[:, :, 0:W], Act.Copy)
            if c % 2 == 1:
                # one store per channel pair, issued from ACT so the SP load
                # queue never waits behind store dependencies
                nc.scalar.dma_start(
                    out_d[b, c - 1:c + 1, :, :].rearrange(
                        "c (s p) w -> p c s w", s=NS),
                    ob[:])

        # ---------------- schedule ----------------
        # head: load + pool sample 0: DVE tensor-scalar accumulators for
        # most tiles, ACT's accumulator for one tile per group plus g3-s1
        # (ACT is otherwise idle in the head), all chasing the SP loads
        def act_pool_tile(b, g, s):
            t = xt[(b, g, s)]
            cs = cspool.tile([KP, GC], f32, tag="cs", name="cs")
            dmp = dpool.tile([KP, W], bf16, tag="dump", name="dump")
            for cc in range(GC):
                nc.scalar.activation(dmp[:], t[:, cc, 1:W + 1],
                                     Act.Copy, accum_out=cs[:, cc:cc + 1])
            csum[(b, g, s)] = cs

        # ACT only pools early-loaded tiles: its in-order queue must be free
        # for the MLP chain the moment the last (DVE-pooled) tile lands
        head_act = {(0, 2), (1, 1), (1, 3), (2, 2)}
        for g in range(NG):
            # alternate HWDGE (SP) and SWDGE (GpSimd) issue: two issue paths
            # keep the DMA pipe at its transfer-limited rate in the head
            emit_load(0, g)
            for s in range(NS):
                if (g, s) in head_act:
                    act_pool_tile(0, g, s)
                else:
                    pool_tile(0, g, s)
        kb0 = emit_mlp(0)

        # conv sample 0; sample 1's loads go out immediately in order
        # g2,g3,g0,g1 (g2/g3 land on fresh slots, g0/g1 self-gate on slot
        # reuse after conv0 drains channels 7/15).  Its pooling interleaves
        # into the DVE stream keyed to conv progress so the in-order stream
        # never stalls on an unloaded tile.  GpSimd builds every 4th
        # channel's band matrices to keep DVE under the PE-window budget.
        for g in (2, 3, 0, 1):
            emit_load(1, g)

        def conv_sample(b, kbs, interleave):
            ob = None
            last_a3 = None
            for g in range(NG):
                a3 = emit_amat_group(kbs, g, halves=2 if g == 0 else 1)
                last_a3 = a3
                for cc in range(GC):
                    c = g * GC + cc
                    if c % 2 == 0:
                        ob = opool.tile([SH, 2, NS, W], bf16, tag="ob",
                                        name="ob")
                    emit_conv_channel(b, c, a3, ob)
            if interleave:
                for g in (2, 3, 0, 1):
                    for s in range(NS):
                        pool_tile(1, g, s)

        conv_sample(0, kb0, True)
        kb1 = emit_mlp(1)
        conv_sample(1, kb1, False)

    nc.compile()
    return nc


def _make_exec():
    """Build + jit the SPMD executable once; returns a callable over numpy inputs."""
    import jax
    from jax.sharding import Mesh, PartitionSpec
    from jax.experimental.shard_map import shard_map
    from concourse import bass2jax
    import concourse.mybir as mybir

    nc = _build_nc()
    _CACHE["nc"] = nc
    bass2jax.install_neuronx_cc_hook()

    in_names, out_names, out_shapes, out_dtypes = [], [], [], []
    for alloc in nc.m.functions[0].allocations:
        if not isinstance(alloc, mybir.MemoryLocationSet):
            continue
        name = alloc.memorylocations[0].name
        if alloc.kind == "ExternalInput":
            in_names.append(name)
        elif alloc.kind == "ExternalOutput":
            out_names.append(name)
            out_shapes.append(tuple(alloc.tensor_shape))
            out_dtypes.append(mybir.dt.np(alloc.dtype))
    partition_name = nc.partition_id_tensor.name if nc.partition_id_tensor else None
    if partition_name in in_names:
        in_names.remove(partition_name)
    n_params = len(in_names)
    out_avals = [jax.core.ShapedArray(s, d) for s, d in zip(out_shapes, out_dtypes)]
    all_names = in_names + out_names
    if partition_name is not None:
        all_names = all_names + [partition_name]
    donate = tuple(range(n_params, n_params + len(out_names)))

    def _body(*args):
        operands = list(args)
        if partition_name is not None:
            operands.append(bass2jax.partition_id_tensor())
        outs = bass2jax._bass_exec_p.bind(
            *operands,
            out_avals=tuple(out_avals),
            in_names=tuple(all_names),
            out_names=tuple(out_names),
            lowering_input_output_aliases=(),
            sim_require_finite=True,
            sim_require_nnan=True,
            nc=nc,
        )
        return tuple(outs)

    devices = jax.devices()[:N_CORES]
    mesh = Mesh(np.asarray(devices), ("core",))
    in_specs = (PartitionSpec("core"),) * (n_params + len(out_names))
    out_specs = (PartitionSpec("core"),) * len(out_names)
    sharded = jax.jit(
        shard_map(_body, mesh=mesh, in_specs=in_specs, out_specs=out_specs,
                  check_rep=False),
        donate_argnums=donate, keep_unused=True)

    def run(in_maps):
        concat_in = [
            np.concatenate([np.asarray(in_maps[c][nm]) for c in range(N_CORES)], axis=0)
            for nm in in_names
        ]
        concat_zeros = [
            np.zeros((N_CORES * s[0], *s[1:]), d)
            for s, d in zip(out_shapes, out_dtypes)
        ]
        out_arrs = sharded(*concat_in, *concat_zeros)
        out_arrs = jax.block_until_ready(out_arrs)
        return {nm: np.asarray(out_arrs[i]) for i, nm in enumerate(out_names)}

    return run


def _run(inputs, trace=False):
    import ml_dtypes
    if "exec" not in _CACHE:
        _CACHE["exec"] = _make_exec()
    run = _CACHE["exec"]

    x16 = np.ascontiguousarray(inputs["x"]).astype(ml_dtypes.bfloat16)
    in_maps = []
    for i in range(N_CORES):
        in_maps.append({
            "x": x16[i * B_PER_CORE:(i + 1) * B_PER_CORE],
            "w1": inputs["w1"], "b1": inputs["b1"],
            "w2": inputs["w2"], "b2": inputs["b2"],
        })
    outs = run(in_maps)
    out = outs["out"].reshape(B, NK, H, W).astype(np.float32)
    return out, None


def kernel(**inputs):
    out, _ = _run(inputs, trace=False)
    return out
